# revision 11
# baseline (speedup 1.0000x reference)
"""CrossAttention (drug x target) Trainium2 Bass kernel.

Sharding: data-parallel over batch, 16 batches -> 8 cores x 2 batches.
Each core: full cross-attention for its 2 batches, fp32 throughout.

Per-batch pipeline (on one core):
  phase 1: PE-transpose target tiles -> tT [c,m]; K^T = Wk^T tT (per
           128-fold), V = tT^T Wv; stage K^T/V to internal DRAM.
  phase 2: Q^T per head (scaled, augmented with a ones row), K^T reloaded
           per-head-aligned (augmented with the column-mask bias row), so
           S = scale*QK^T + colmask in one matmul; row mask applied as a
           per-partition multiply (a fully-masked row becomes constant ->
           uniform softmax, matching the reference exactly); softmax with
           constant -8 shift (shift-invariant); PE-transpose the probs;
           PV per head; out-proj accumulated over heads in PSUM;
           +bias +residual, LayerNorm; DMA out.
"""

from contextlib import ExitStack

import numpy as np

import concourse.bass as bass
import concourse.mybir as mybir
import concourse.tile as tile
from concourse.bass_utils import run_bass_kernel_spmd
from concourse.masks import make_identity

F32 = mybir.dt.float32
I32 = mybir.dt.int32
AF = mybir.ActivationFunctionType
ALU = mybir.AluOpType

B, N, M = 16, 256, 1024
CD, CT = 768, 2560            # drug dim, target dim
H, DH, INNER = 8, 96, 768
AUG = DH + 1                  # augmented contraction (mask row)
P = 128
KD, KC = CD // P, CT // P     # 6, 20 contraction folds
NT = N // P                   # 2 n-tiles
MF = M // P                   # 8 m-folds
BPC = 2                       # batches per core
NCORES = 8
SCALE = float(DH) ** -0.5
LN_EPS = 1e-5
EXP_SHIFT = -8.0
MT = 256                      # phase-1 m-tile
NMT = M // MT                 # 4
CH = CT // 2                  # phase-1 c-half


def build_bass():
    nc = bass.Bass()
    drug = nc.declare_dram_parameter("drug", [BPC, N, CD], F32, isOutput=False)
    target = nc.declare_dram_parameter("target", [BPC, M, CT], F32, isOutput=False)
    dmask = nc.declare_dram_parameter("dmask", [BPC, N], I32, isOutput=False)
    pmask = nc.declare_dram_parameter("pmask", [BPC, M], I32, isOutput=False)
    Wq = nc.declare_dram_parameter("Wq", [CD, INNER], F32, isOutput=False)
    Wk = nc.declare_dram_parameter("Wk", [CT, INNER], F32, isOutput=False)
    Wv = nc.declare_dram_parameter("Wv", [CT, INNER], F32, isOutput=False)
    Wo = nc.declare_dram_parameter("Wo", [INNER, CD], F32, isOutput=False)
    bo = nc.declare_dram_parameter("bo", [CD], F32, isOutput=False)
    gamma = nc.declare_dram_parameter("gamma", [CD], F32, isOutput=False)
    beta = nc.declare_dram_parameter("beta", [CD], F32, isOutput=False)
    out = nc.declare_dram_parameter("out", [BPC, N, CD], F32, isOutput=True)
    attn = nc.declare_dram_parameter("attn", [BPC, H, N, M], F32, isOutput=True)

    Kst = nc.dram_tensor("Kst", [BPC, INNER, M], F32)
    Vst = nc.dram_tensor("Vst", [BPC, M, INNER], F32)

    with tile.TileContext(nc) as tc, ExitStack() as stack:
        cpool = stack.enter_context(tc.tile_pool(name="const", bufs=1))
        ident = cpool.tile([P, P], F32)
        make_identity(nc, ident[:])
        shift_t = cpool.tile([P, 1], F32, tag="shift")
        nc.vector.memset(shift_t[:], EXP_SHIFT)
        eps_t = cpool.tile([P, 1], F32, tag="eps")
        nc.vector.memset(eps_t[:], LN_EPS)

        # ---------------- phase 1: K/V projections ----------------
        with tc.tile_pool(name="wkv", bufs=1) as wpool, \
             tc.tile_pool(name="p1nat", bufs=2) as p1nat, \
             tc.tile_pool(name="p1tt", bufs=1) as p1tt, \
             tc.tile_pool(name="p1ev", bufs=3) as p1ev, \
             tc.tile_pool(name="p1tp", bufs=2, space="PSUM") as p1tp, \
             tc.tile_pool(name="p1mm", bufs=2, space="PSUM") as p1mm:
            wk_sb = wpool.tile([P, KC, INNER], F32)
            nc.sync.dma_start(out=wk_sb[:], in_=Wk.rearrange("(ko p) c -> p ko c", p=P))
            wv_sb = wpool.tile([P, KC, INNER], F32)
            nc.sync.dma_start(out=wv_sb[:], in_=Wv.rearrange("(ko p) c -> p ko c", p=P))

            for b in range(BPC):
                for mt in range(NMT):
                    m0 = mt * MT
                    tT = p1tt.tile([P, KC, MT], F32, tag="tT")
                    for ch in range(2):
                        nat = p1nat.tile([P, MT // P, CH], F32, tag="nat")
                        nc.sync.dma_start(
                            out=nat[:],
                            in_=target[b, m0:m0 + MT, ch * CH:(ch + 1) * CH]
                            .rearrange("(s p) c -> p s c", p=P))
                        for kk in range(KC // 2):
                            k = ch * (KC // 2) + kk
                            for s in range(MT // P):
                                ptp = p1tp.tile([P, P], F32, tag="tp")
                                nc.tensor.transpose(
                                    ptp[:], nat[:, s, kk * P:(kk + 1) * P], ident[:])
                                nc.vector.tensor_copy(
                                    out=tT[:, k, s * P:(s + 1) * P], in_=ptp[:])
                    # K^T tiles: [inner-fold 128, MT]
                    for i in range(KD):
                        pk = p1mm.tile([P, MT], F32, tag="pk")
                        for k in range(KC):
                            nc.tensor.matmul(
                                pk[:], wk_sb[:, k, i * P:(i + 1) * P], tT[:, k, :],
                                start=(k == 0), stop=(k == KC - 1))
                        sk = p1ev.tile([P, MT], F32, tag="sk")
                        nc.vector.tensor_copy(out=sk[:], in_=pk[:])
                        nc.sync.dma_start(
                            out=Kst[b, i * P:(i + 1) * P, m0:m0 + MT], in_=sk[:])
                    # V tiles: [m 128, 384]
                    for s in range(MT // P):
                        for j in range(2):
                            pv = p1mm.tile([P, 384], F32, tag="pv")
                            for k in range(KC):
                                nc.tensor.matmul(
                                    pv[:], tT[:, k, s * P:(s + 1) * P],
                                    wv_sb[:, k, j * 384:(j + 1) * 384],
                                    start=(k == 0), stop=(k == KC - 1))
                            sv = p1ev.tile([P, 384], F32, tag="sv")
                            nc.vector.tensor_copy(out=sv[:], in_=pv[:])
                            nc.sync.dma_start(
                                out=Vst[b, m0 + s * P:m0 + (s + 1) * P,
                                        j * 384:(j + 1) * 384],
                                in_=sv[:])

        # ---------------- phase 2: attention ----------------
        with tc.tile_pool(name="p2w", bufs=1) as p2w, \
             tc.tile_pool(name="p2sb", bufs=2) as p2sb, \
             tc.tile_pool(name="p2ps", bufs=1, space="PSUM") as p2ps, \
             tc.tile_pool(name="p2ps2", bufs=2, space="PSUM") as p2ps2, \
             tc.tile_pool(name="p2pt", bufs=2, space="PSUM") as p2pt:
            # broadcast bo/gamma/beta across partitions via rank-1 matmul
            gbb = p2w.tile([P, 3 * CD], F32)
            with tc.tile_pool(name="bcs", bufs=1) as bcs:
                gbb_row = bcs.tile([1, 3 * CD], F32)
                nc.sync.dma_start(out=gbb_row[0:1, 0:CD], in_=bo[None, :])
                nc.sync.dma_start(out=gbb_row[0:1, CD:2 * CD], in_=gamma[None, :])
                nc.sync.dma_start(out=gbb_row[0:1, 2 * CD:], in_=beta[None, :])
                ones1 = bcs.tile([1, P], F32)
                nc.vector.memset(ones1[:], 1.0)
                for j in range(0, 3 * CD, 384):
                    pt_full = p2ps2.tile([P, 512], F32, tag="ps_s", name="pt_full")
                    pt = pt_full[:, :384]
                    nc.tensor.matmul(pt[:], ones1[:], gbb_row[0:1, j:j + 384],
                                     start=True, stop=True)
                    nc.vector.tensor_copy(out=gbb[:, j:j + 384], in_=pt[:])
            bo_b = gbb[:, 0:CD]
            gamma_b = gbb[:, CD:2 * CD]
            beta_b = gbb[:, 2 * CD:]

            # masks
            rm = []       # per batch [128, NT] f32 row mask
            colneg = []   # per batch [1, M] f32 column bias (-1e6 on masked)
            with tc.tile_pool(name="msc", bufs=1) as msc:
                for b in range(BPC):
                    rmi = msc.tile([P, NT], I32, tag=f"rmi{b}")
                    nc.sync.dma_start(out=rmi[:],
                                      in_=dmask[b].rearrange("(t p) -> p t", p=P))
                    rmf = p2w.tile([P, NT], F32, tag=f"rmf{b}")
                    nc.vector.tensor_copy(out=rmf[:], in_=rmi[:])
                    rm.append(rmf)
                    pmi = msc.tile([1, M], I32, tag=f"pmi{b}")
                    nc.sync.dma_start(out=pmi[:], in_=pmask[b][None, :])
                    pmf = msc.tile([1, M], F32, tag=f"pmf{b}")
                    nc.vector.tensor_copy(out=pmf[:], in_=pmi[:])
                    cn = p2w.tile([1, M], F32, tag=f"cn{b}")
                    nc.vector.tensor_scalar(out=cn[:], in0=pmf[:], scalar1=1.0e6,
                                            scalar2=-1.0e6, op0=ALU.mult,
                                            op1=ALU.add)
                    colneg.append(cn)

            wq_sb = p2w.tile([P, KD, INNER], F32)
            nc.sync.dma_start(out=wq_sb[:], in_=Wq.rearrange("(ko p) c -> p ko c", p=P))
            wo_sb = p2w.tile([P, H, CD], F32)
            for h in range(H):
                nc.sync.dma_start(out=wo_sb[0:DH, h, :],
                                  in_=Wo[h * DH:(h + 1) * DH, :])

            for b in range(BPC):
                kt = p2sb.tile([P, H, M], F32, tag="kt", bufs=1)
                for h in range(H):
                    nc.sync.dma_start(out=kt[0:DH, h, :],
                                      in_=Kst[b, h * DH:(h + 1) * DH, :])
                    nc.sync.dma_start(out=kt[DH:AUG, h, :], in_=colneg[b][0:1, :])
                v_sb = p2sb.tile([P, MF, INNER], F32, tag="v", bufs=1)
                nc.sync.dma_start(out=v_sb[:],
                                  in_=Vst[b].rearrange("(mf p) c -> p mf c", p=P))
                drug_sb = p2sb.tile([P, NT, CD], F32, tag="drug", bufs=1)
                nc.sync.dma_start(out=drug_sb[:],
                                  in_=drug[b].rearrange("(t p) c -> p t c", p=P))
                drugT = p2sb.tile([P, KD, N], F32, tag="drugT", bufs=1)
                for k in range(KD):
                    for t in range(NT):
                        ptp = p2pt.tile([P, P], F32, tag="tp2")
                        nc.tensor.transpose(
                            ptp[:], drug_sb[:, t, k * P:(k + 1) * P], ident[:])
                        nc.vector.tensor_copy(
                            out=drugT[:, k, t * P:(t + 1) * P], in_=ptp[:])
                qt = p2sb.tile([P, H, N], F32, tag="qt", bufs=1)
                for h in range(H):
                    pq = p2ps.tile([P, N], F32, tag="pq")
                    for k in range(KD):
                        nc.tensor.matmul(
                            pq[0:DH, :], wq_sb[:, k, h * DH:(h + 1) * DH],
                            drugT[:, k, :], start=(k == 0), stop=(k == KD - 1))
                    nc.scalar.activation(qt[0:DH, h, :], pq[0:DH, :], AF.Copy,
                                         scale=SCALE)
                    nc.vector.memset(qt[DH:AUG, h, :], 1.0)

                ot = p2sb.tile([P, H, N], F32, tag="ot", bufs=1)
                for h in range(H):
                    attnT = p2sb.tile([P, MF, N], F32, tag="attnT", bufs=1)
                    for t in range(NT):
                        s_sb = p2sb.tile([P, M], F32, tag="s_sb")
                        for mh in range(2):
                            ps_s = p2ps2.tile([P, 512], F32, tag="ps_s")
                            nc.tensor.matmul(
                                ps_s[:], qt[0:AUG, h, t * P:(t + 1) * P],
                                kt[0:AUG, h, mh * 512:(mh + 1) * 512],
                                start=True, stop=True)
                            nc.vector.tensor_scalar(
                                out=s_sb[:, mh * 512:(mh + 1) * 512], in0=ps_s[:],
                                scalar1=rm[b][:, t:t + 1], scalar2=None,
                                op0=ALU.mult)
                        sum_e = p2sb.tile([P, 1], F32, tag="sum_e")
                        nc.scalar.activation(s_sb[:], s_sb[:], AF.Exp,
                                             bias=shift_t[:], accum_out=sum_e[:])
                        rs = p2sb.tile([P, 1], F32, tag="rs")
                        nc.vector.reciprocal(rs[:], sum_e[:])
                        nc.vector.tensor_scalar(
                            out=s_sb[:], in0=s_sb[:], scalar1=rs[:],
                            scalar2=None, op0=ALU.mult)
                        nc.sync.dma_start(
                            out=attn[b, h, t * P:(t + 1) * P, :], in_=s_sb[:])
                        for mf in range(MF):
                            ptp = p2pt.tile([P, P], F32, tag="tp2")
                            nc.tensor.transpose(
                                ptp[:], s_sb[:, mf * P:(mf + 1) * P], ident[:])
                            nc.vector.tensor_copy(
                                out=attnT[:, mf, t * P:(t + 1) * P], in_=ptp[:])
                    po = p2ps.tile([P, N], F32, tag="po")
                    for mf in range(MF):
                        nc.tensor.matmul(
                            po[0:DH, :], v_sb[:, mf, h * DH:(h + 1) * DH],
                            attnT[:, mf, :], start=(mf == 0), stop=(mf == MF - 1))
                    nc.vector.tensor_copy(out=ot[0:DH, h, :], in_=po[0:DH, :])

                # out-proj + bias + residual + LayerNorm
                for t in range(NT):
                    x_sb = p2sb.tile([P, CD], F32, tag="x_sb", bufs=1)
                    for j in range(2):
                        pj = p2ps.tile([P, 384], F32, tag="pj")
                        for h in range(H):
                            nc.tensor.matmul(
                                pj[:], ot[0:DH, h, t * P:(t + 1) * P],
                                wo_sb[0:DH, h, j * 384:(j + 1) * 384],
                                start=(h == 0), stop=(h == H - 1))
                        nc.vector.tensor_tensor(
                            out=x_sb[:, j * 384:(j + 1) * 384], in0=pj[:],
                            in1=drug_sb[:, t, j * 384:(j + 1) * 384], op=ALU.add)
                    nc.vector.tensor_tensor(out=x_sb[:], in0=x_sb[:], in1=bo_b,
                                            op=ALU.add)
                    mu = p2sb.tile([P, 1], F32, tag="mu")
                    nc.vector.reduce_sum(out=mu[:], in_=x_sb[:],
                                         axis=mybir.AxisListType.X)
                    nc.vector.tensor_scalar(out=mu[:], in0=mu[:],
                                            scalar1=1.0 / CD, scalar2=None,
                                            op0=ALU.mult)
                    xc = p2sb.tile([P, CD], F32, tag="xc", bufs=1)
                    nc.vector.tensor_scalar(out=xc[:], in0=x_sb[:],
                                            scalar1=mu[:], scalar2=None,
                                            op0=ALU.subtract)
                    sq = p2sb.tile([P, CD], F32, tag="sq", bufs=1)
                    nc.vector.tensor_tensor(out=sq[:], in0=xc[:], in1=xc[:],
                                            op=ALU.mult)
                    var = p2sb.tile([P, 1], F32, tag="var")
                    nc.vector.reduce_sum(out=var[:], in_=sq[:],
                                         axis=mybir.AxisListType.X)
                    nc.vector.tensor_scalar(out=var[:], in0=var[:],
                                            scalar1=1.0 / CD, scalar2=None,
                                            op0=ALU.mult)
                    sd = p2sb.tile([P, 1], F32, tag="sd")
                    nc.scalar.activation(sd[:], var[:], AF.Sqrt, bias=eps_t[:])
                    rstd = p2sb.tile([P, 1], F32, tag="rstd")
                    nc.vector.reciprocal(rstd[:], sd[:])
                    y = p2sb.tile([P, CD], F32, tag="y", bufs=1)
                    nc.vector.tensor_scalar(out=y[:], in0=xc[:], scalar1=rstd[:],
                                            scalar2=None, op0=ALU.mult)
                    nc.vector.tensor_tensor(out=y[:], in0=y[:], in1=gamma_b,
                                            op=ALU.mult)
                    nc.vector.tensor_tensor(out=y[:], in0=y[:], in1=beta_b,
                                            op=ALU.add)
                    nc.sync.dma_start(out=out[b, t * P:(t + 1) * P, :], in_=y[:])

    _split_matmul_waits(nc)
    return nc


def _split_matmul_waits(nc):
    """Walrus allows only one sync-wait per TPB instruction (two on event
    semaphores). Hoist excess waits onto same-engine NoOps inserted before."""
    for fn in nc.m.functions:
        blocks = getattr(fn, "blocks", None)
        ilists = [b.instructions for b in blocks] if blocks else [fn.instructions]
        for ilist in ilists:
            new = []
            for inst in ilist:
                if not isinstance(inst, (mybir.InstNoOp, mybir.InstEventSemaphore)):
                    si = getattr(inst, "sync_info", None)
                    if si is not None and si.on_wait and len(si.on_wait) > 1:
                        extra, keep = list(si.on_wait[:-1]), [si.on_wait[-1]]
                        si.on_wait = keep
                        for w in extra:
                            nop = mybir.InstNoOp(
                                name=nc.get_next_instruction_name(),
                                sync_info=mybir.SyncInfo(on_wait=[w], on_update=[]),
                                engine=inst.engine,
                                bass_nofuse=True,
                            )
                            nc.register_instruction(nop)
                            new.append(nop)
                new.append(inst)
            ilist[:] = new


_NC = None


def kernel(drug, target, drug_mask, pro_mask, Wq, Wk, Wv, Wo, bo, gamma, beta,
           **extra):
    global _NC
    if _NC is None:
        _NC = build_bass()
    nc = _NC
    in_maps = []
    for c in range(NCORES):
        sl = slice(c * BPC, (c + 1) * BPC)
        in_maps.append({
            "drug": np.ascontiguousarray(drug[sl], dtype=np.float32),
            "target": np.ascontiguousarray(target[sl], dtype=np.float32),
            "dmask": np.ascontiguousarray(drug_mask[sl], dtype=np.int32),
            "pmask": np.ascontiguousarray(pro_mask[sl], dtype=np.int32),
            "Wq": np.ascontiguousarray(Wq, dtype=np.float32),
            "Wk": np.ascontiguousarray(Wk, dtype=np.float32),
            "Wv": np.ascontiguousarray(Wv, dtype=np.float32),
            "Wo": np.ascontiguousarray(Wo, dtype=np.float32),
            "bo": np.ascontiguousarray(bo, dtype=np.float32),
            "gamma": np.ascontiguousarray(gamma, dtype=np.float32),
            "beta": np.ascontiguousarray(beta, dtype=np.float32),
        })
    res = run_bass_kernel_spmd(nc, in_maps, list(range(NCORES))).results
    out = np.concatenate([r["out"] for r in res], axis=0)
    attn = np.concatenate([r["attn"] for r in res], axis=0)
    return out, attn


# revision 14
# speedup vs baseline: 1.0765x; 1.0765x over previous
"""CrossAttention (drug x target) Trainium2 Bass kernel.

Sharding: data-parallel over batch, 16 batches -> 8 cores x 2 batches.
Each core: full cross-attention for its 2 batches, fp32 throughout.

Per-batch pipeline (on one core):
  phase 1: PE-transpose target tiles -> tT [c,m]; K^T = Wk^T tT (per
           128-fold), V = tT^T Wv; stage K^T/V to internal DRAM.
  phase 2: Q^T per head (scaled, augmented with a ones row), K^T reloaded
           per-head-aligned (augmented with the column-mask bias row), so
           S = scale*QK^T + colmask in one matmul; row mask applied as a
           per-partition multiply (a fully-masked row becomes constant ->
           uniform softmax, matching the reference exactly); softmax with
           constant -8 shift (shift-invariant); PE-transpose the probs;
           PV per head; out-proj accumulated over heads in PSUM;
           +bias +residual, LayerNorm; DMA out.
"""

from contextlib import ExitStack

import numpy as np

import concourse.bass as bass
import concourse.mybir as mybir
import concourse.tile as tile
from concourse.bass_utils import run_bass_kernel_spmd
from concourse.masks import make_identity

F32 = mybir.dt.float32
F32R = mybir.dt.float32r
I32 = mybir.dt.int32
AF = mybir.ActivationFunctionType
ALU = mybir.AluOpType

B, N, M = 16, 256, 1024
CD, CT = 768, 2560            # drug dim, target dim
H, DH, INNER = 8, 96, 768
AUG = DH + 1                  # augmented contraction (mask row)
P = 128
KD, KC = CD // P, CT // P     # 6, 20 contraction folds
NT = N // P                   # 2 n-tiles
MF = M // P                   # 8 m-folds
BPC = 2                       # batches per core
NCORES = 8
SCALE = float(DH) ** -0.5
LN_EPS = 1e-5
EXP_SHIFT = -8.0
MT = 256                      # phase-1 m-tile
NMT = M // MT                 # 4
CH = CT // 2                  # phase-1 c-half


def build_bass():
    nc = bass.Bass()
    drug = nc.declare_dram_parameter("drug", [BPC, N, CD], F32, isOutput=False)
    target = nc.declare_dram_parameter("target", [BPC, M, CT], F32, isOutput=False)
    dmask = nc.declare_dram_parameter("dmask", [BPC, N], I32, isOutput=False)
    pmask = nc.declare_dram_parameter("pmask", [BPC, M], I32, isOutput=False)
    Wq = nc.declare_dram_parameter("Wq", [CD, INNER], F32, isOutput=False)
    Wk = nc.declare_dram_parameter("Wk", [CT, INNER], F32, isOutput=False)
    Wv = nc.declare_dram_parameter("Wv", [CT, INNER], F32, isOutput=False)
    Wo = nc.declare_dram_parameter("Wo", [INNER, CD], F32, isOutput=False)
    bo = nc.declare_dram_parameter("bo", [CD], F32, isOutput=False)
    gamma = nc.declare_dram_parameter("gamma", [CD], F32, isOutput=False)
    beta = nc.declare_dram_parameter("beta", [CD], F32, isOutput=False)
    out = nc.declare_dram_parameter("out", [BPC, N, CD], F32, isOutput=True)
    attn = nc.declare_dram_parameter("attn", [BPC, H, N, M], F32, isOutput=True)

    Kst = nc.dram_tensor("Kst", [BPC, INNER, M], F32)
    Vst = nc.dram_tensor("Vst", [BPC, M, INNER], F32)

    with tile.TileContext(nc) as tc, ExitStack() as stack:
        cpool = stack.enter_context(tc.tile_pool(name="const", bufs=1))
        ident = cpool.tile([P, P], F32)
        make_identity(nc, ident[:])
        shift_t = cpool.tile([P, 1], F32, tag="shift")
        nc.vector.memset(shift_t[:], EXP_SHIFT)
        eps_t = cpool.tile([P, 1], F32, tag="eps")
        nc.vector.memset(eps_t[:], LN_EPS)

        # ---------------- phase 1: K/V projections ----------------
        with tc.tile_pool(name="wkv", bufs=1) as wpool, \
             tc.tile_pool(name="p1nat", bufs=2) as p1nat, \
             tc.tile_pool(name="p1tt", bufs=1) as p1tt, \
             tc.tile_pool(name="p1ev", bufs=3) as p1ev, \
             tc.tile_pool(name="p1tp", bufs=2, space="PSUM") as p1tp, \
             tc.tile_pool(name="p1mm", bufs=2, space="PSUM") as p1mm:
            wk_sb = wpool.tile([P, KC, INNER], F32R)
            nc.gpsimd.dma_start(out=wk_sb[:], in_=Wk.rearrange("(ko p) c -> p ko c", p=P))
            wv_sb = wpool.tile([P, KC, INNER], F32R)
            nc.gpsimd.dma_start(out=wv_sb[:], in_=Wv.rearrange("(ko p) c -> p ko c", p=P))

            for b in range(BPC):
                for mt in range(NMT):
                    m0 = mt * MT
                    tT = p1tt.tile([P, KC, MT], F32R, tag="tT")
                    for ch in range(2):
                        nat = p1nat.tile([P, MT // P, CH], F32, tag="nat")
                        nc.sync.dma_start(
                            out=nat[:],
                            in_=target[b, m0:m0 + MT, ch * CH:(ch + 1) * CH]
                            .rearrange("(s p) c -> p s c", p=P))
                        for kk in range(KC // 2):
                            k = ch * (KC // 2) + kk
                            for s in range(MT // P):
                                ptp = p1tp.tile([P, P], F32, tag="tp")
                                nc.tensor.transpose(
                                    ptp[:], nat[:, s, kk * P:(kk + 1) * P], ident[:])
                                nc.vector.tensor_copy(
                                    out=tT[:, k, s * P:(s + 1) * P], in_=ptp[:])
                    # K^T tiles: [inner-fold 128, MT]
                    for i in range(KD):
                        pk = p1mm.tile([P, MT], F32, tag="pk")
                        for k in range(KC):
                            nc.tensor.matmul(
                                pk[:], wk_sb[:, k, i * P:(i + 1) * P], tT[:, k, :],
                                start=(k == 0), stop=(k == KC - 1))
                        sk = p1ev.tile([P, MT], F32, tag="sk")
                        nc.vector.tensor_copy(out=sk[:], in_=pk[:])
                        nc.sync.dma_start(
                            out=Kst[b, i * P:(i + 1) * P, m0:m0 + MT], in_=sk[:])
                    # V tiles: [m 128, 384]
                    for s in range(MT // P):
                        for j in range(2):
                            pv = p1mm.tile([P, 384], F32, tag="pv")
                            for k in range(KC):
                                nc.tensor.matmul(
                                    pv[:], tT[:, k, s * P:(s + 1) * P],
                                    wv_sb[:, k, j * 384:(j + 1) * 384],
                                    start=(k == 0), stop=(k == KC - 1))
                            sv = p1ev.tile([P, 384], F32, tag="sv")
                            nc.vector.tensor_copy(out=sv[:], in_=pv[:])
                            nc.sync.dma_start(
                                out=Vst[b, m0 + s * P:m0 + (s + 1) * P,
                                        j * 384:(j + 1) * 384],
                                in_=sv[:])

        # ---------------- phase 2: attention ----------------
        with tc.tile_pool(name="p2w", bufs=1) as p2w, \
             tc.tile_pool(name="p2sb", bufs=2) as p2sb, \
             tc.tile_pool(name="p2ps", bufs=1, space="PSUM") as p2ps, \
             tc.tile_pool(name="p2ps2", bufs=2, space="PSUM") as p2ps2, \
             tc.tile_pool(name="p2pt", bufs=2, space="PSUM") as p2pt:
            # broadcast bo/gamma/beta across partitions via rank-1 matmul
            gbb = p2w.tile([P, 3 * CD], F32)
            with tc.tile_pool(name="bcs", bufs=1) as bcs:
                gbb_row = bcs.tile([1, 3 * CD], F32)
                nc.sync.dma_start(out=gbb_row[0:1, 0:CD], in_=bo[None, :])
                nc.sync.dma_start(out=gbb_row[0:1, CD:2 * CD], in_=gamma[None, :])
                nc.sync.dma_start(out=gbb_row[0:1, 2 * CD:], in_=beta[None, :])
                ones1 = bcs.tile([1, P], F32)
                nc.vector.memset(ones1[:], 1.0)
                for j in range(0, 3 * CD, 384):
                    pt_full = p2ps2.tile([P, 512], F32, tag="ps_s", name="pt_full")
                    pt = pt_full[:, :384]
                    nc.tensor.matmul(pt[:], ones1[:], gbb_row[0:1, j:j + 384],
                                     start=True, stop=True)
                    nc.vector.tensor_copy(out=gbb[:, j:j + 384], in_=pt[:])
            bo_b = gbb[:, 0:CD]
            gamma_b = gbb[:, CD:2 * CD]
            beta_b = gbb[:, 2 * CD:]

            # masks
            rm = []       # per batch [128, NT] f32 row mask
            colneg = []   # per batch [1, M] f32 column bias (-1e6 on masked)
            with tc.tile_pool(name="msc", bufs=1) as msc:
                for b in range(BPC):
                    rmi = msc.tile([P, NT], I32, tag=f"rmi{b}")
                    nc.sync.dma_start(out=rmi[:],
                                      in_=dmask[b].rearrange("(t p) -> p t", p=P))
                    rmf = p2w.tile([P, NT], F32, tag=f"rmf{b}")
                    nc.vector.tensor_copy(out=rmf[:], in_=rmi[:])
                    rm.append(rmf)
                    pmi = msc.tile([1, M], I32, tag=f"pmi{b}")
                    nc.sync.dma_start(out=pmi[:], in_=pmask[b][None, :])
                    pmf = msc.tile([1, M], F32, tag=f"pmf{b}")
                    nc.vector.tensor_copy(out=pmf[:], in_=pmi[:])
                    cn = p2w.tile([1, M], F32, tag=f"cn{b}")
                    nc.vector.tensor_scalar(out=cn[:], in0=pmf[:], scalar1=1.0e6,
                                            scalar2=-1.0e6, op0=ALU.mult,
                                            op1=ALU.add)
                    colneg.append(cn)

            wq_sb = p2w.tile([P, KD, INNER], F32R)
            nc.gpsimd.dma_start(out=wq_sb[:], in_=Wq.rearrange("(ko p) c -> p ko c", p=P))
            wo_sb = p2w.tile([P, H, CD], F32R)
            for h in range(H):
                nc.gpsimd.dma_start(out=wo_sb[0:DH, h, :],
                                  in_=Wo[h * DH:(h + 1) * DH, :])

            for b in range(BPC):
                kt = p2sb.tile([P, H, M], F32, tag="kt", bufs=1)
                for h in range(H):
                    nc.sync.dma_start(out=kt[0:DH, h, :],
                                      in_=Kst[b, h * DH:(h + 1) * DH, :])
                    nc.sync.dma_start(out=kt[DH:AUG, h, :], in_=colneg[b][0:1, :])
                v_sb = p2sb.tile([P, MF, INNER], F32R, tag="v", bufs=1)
                nc.gpsimd.dma_start(out=v_sb[:],
                                  in_=Vst[b].rearrange("(mf p) c -> p mf c", p=P))
                drug_sb = p2sb.tile([P, NT, CD], F32, tag="drug", bufs=1)
                nc.sync.dma_start(out=drug_sb[:],
                                  in_=drug[b].rearrange("(t p) c -> p t c", p=P))
                drugT = p2sb.tile([P, KD, N], F32R, tag="drugT", bufs=1)
                for k in range(KD):
                    for t in range(NT):
                        ptp = p2pt.tile([P, P], F32, tag="tp2")
                        nc.tensor.transpose(
                            ptp[:], drug_sb[:, t, k * P:(k + 1) * P], ident[:])
                        nc.vector.tensor_copy(
                            out=drugT[:, k, t * P:(t + 1) * P], in_=ptp[:])
                qt = p2sb.tile([P, H, N], F32, tag="qt", bufs=1)
                for h in range(H):
                    pq = p2ps.tile([P, N], F32, tag="pq")
                    for k in range(KD):
                        nc.tensor.matmul(
                            pq[0:DH, :], wq_sb[:, k, h * DH:(h + 1) * DH],
                            drugT[:, k, :], start=(k == 0), stop=(k == KD - 1))
                    nc.scalar.activation(qt[0:DH, h, :], pq[0:DH, :], AF.Copy,
                                         scale=SCALE)
                    nc.vector.memset(qt[DH:AUG, h, :], 1.0)

                ot = p2sb.tile([P, H, N], F32R, tag="ot", bufs=1)
                for h in range(H):
                    attnT = p2sb.tile([P, MF, N], F32R, tag="attnT", bufs=1)
                    for t in range(NT):
                        s_sb = p2sb.tile([P, M], F32, tag="s_sb")
                        for mh in range(2):
                            ps_s = p2ps2.tile([P, 512], F32, tag="ps_s")
                            nc.tensor.matmul(
                                ps_s[:], qt[0:AUG, h, t * P:(t + 1) * P],
                                kt[0:AUG, h, mh * 512:(mh + 1) * 512],
                                start=True, stop=True)
                            nc.vector.tensor_scalar(
                                out=s_sb[:, mh * 512:(mh + 1) * 512], in0=ps_s[:],
                                scalar1=rm[b][:, t:t + 1], scalar2=None,
                                op0=ALU.mult)
                        sum_e = p2sb.tile([P, 1], F32, tag="sum_e")
                        nc.scalar.activation(s_sb[:], s_sb[:], AF.Exp,
                                             bias=shift_t[:], accum_out=sum_e[:])
                        rs = p2sb.tile([P, 1], F32, tag="rs")
                        nc.vector.reciprocal(rs[:], sum_e[:])
                        nc.vector.tensor_scalar(
                            out=s_sb[:], in0=s_sb[:], scalar1=rs[:],
                            scalar2=None, op0=ALU.mult)
                        nc.sync.dma_start(
                            out=attn[b, h, t * P:(t + 1) * P, :], in_=s_sb[:])
                        for mf in range(MF):
                            ptp = p2pt.tile([P, P], F32, tag="tp2")
                            nc.tensor.transpose(
                                ptp[:], s_sb[:, mf * P:(mf + 1) * P], ident[:])
                            nc.vector.tensor_copy(
                                out=attnT[:, mf, t * P:(t + 1) * P], in_=ptp[:])
                    po = p2ps.tile([P, N], F32, tag="po")
                    for mf in range(MF):
                        nc.tensor.matmul(
                            po[0:DH, :], v_sb[:, mf, h * DH:(h + 1) * DH],
                            attnT[:, mf, :], start=(mf == 0), stop=(mf == MF - 1))
                    nc.vector.tensor_copy(out=ot[0:DH, h, :], in_=po[0:DH, :])

                # out-proj + bias + residual + LayerNorm
                for t in range(NT):
                    x_sb = p2sb.tile([P, CD], F32, tag="x_sb", bufs=1)
                    for j in range(2):
                        pj = p2ps.tile([P, 384], F32, tag="pj")
                        for h in range(H):
                            nc.tensor.matmul(
                                pj[:], ot[0:DH, h, t * P:(t + 1) * P],
                                wo_sb[0:DH, h, j * 384:(j + 1) * 384],
                                start=(h == 0), stop=(h == H - 1))
                        nc.vector.tensor_tensor(
                            out=x_sb[:, j * 384:(j + 1) * 384], in0=pj[:],
                            in1=drug_sb[:, t, j * 384:(j + 1) * 384], op=ALU.add)
                    nc.vector.tensor_tensor(out=x_sb[:], in0=x_sb[:], in1=bo_b,
                                            op=ALU.add)
                    mu = p2sb.tile([P, 1], F32, tag="mu")
                    nc.vector.reduce_sum(out=mu[:], in_=x_sb[:],
                                         axis=mybir.AxisListType.X)
                    nc.vector.tensor_scalar(out=mu[:], in0=mu[:],
                                            scalar1=1.0 / CD, scalar2=None,
                                            op0=ALU.mult)
                    xc = p2sb.tile([P, CD], F32, tag="xc", bufs=1)
                    nc.vector.tensor_scalar(out=xc[:], in0=x_sb[:],
                                            scalar1=mu[:], scalar2=None,
                                            op0=ALU.subtract)
                    sq = p2sb.tile([P, CD], F32, tag="sq", bufs=1)
                    nc.vector.tensor_tensor(out=sq[:], in0=xc[:], in1=xc[:],
                                            op=ALU.mult)
                    var = p2sb.tile([P, 1], F32, tag="var")
                    nc.vector.reduce_sum(out=var[:], in_=sq[:],
                                         axis=mybir.AxisListType.X)
                    nc.vector.tensor_scalar(out=var[:], in0=var[:],
                                            scalar1=1.0 / CD, scalar2=None,
                                            op0=ALU.mult)
                    sd = p2sb.tile([P, 1], F32, tag="sd")
                    nc.scalar.activation(sd[:], var[:], AF.Sqrt, bias=eps_t[:])
                    rstd = p2sb.tile([P, 1], F32, tag="rstd")
                    nc.vector.reciprocal(rstd[:], sd[:])
                    y = p2sb.tile([P, CD], F32, tag="y", bufs=1)
                    nc.vector.tensor_scalar(out=y[:], in0=xc[:], scalar1=rstd[:],
                                            scalar2=None, op0=ALU.mult)
                    nc.vector.tensor_tensor(out=y[:], in0=y[:], in1=gamma_b,
                                            op=ALU.mult)
                    nc.vector.tensor_tensor(out=y[:], in0=y[:], in1=beta_b,
                                            op=ALU.add)
                    nc.sync.dma_start(out=out[b, t * P:(t + 1) * P, :], in_=y[:])

    _split_matmul_waits(nc)
    return nc


def _split_matmul_waits(nc):
    """Walrus allows only one sync-wait per TPB instruction (two on event
    semaphores). Hoist excess waits onto same-engine NoOps inserted before."""
    for fn in nc.m.functions:
        blocks = getattr(fn, "blocks", None)
        ilists = [b.instructions for b in blocks] if blocks else [fn.instructions]
        for ilist in ilists:
            new = []
            for inst in ilist:
                if not isinstance(inst, (mybir.InstNoOp, mybir.InstEventSemaphore)):
                    si = getattr(inst, "sync_info", None)
                    if si is not None and si.on_wait and len(si.on_wait) > 1:
                        extra, keep = list(si.on_wait[:-1]), [si.on_wait[-1]]
                        si.on_wait = keep
                        for w in extra:
                            nop = mybir.InstNoOp(
                                name=nc.get_next_instruction_name(),
                                sync_info=mybir.SyncInfo(on_wait=[w], on_update=[]),
                                engine=inst.engine,
                                bass_nofuse=True,
                            )
                            nc.register_instruction(nop)
                            new.append(nop)
                new.append(inst)
            ilist[:] = new


_NC = None


def kernel(drug, target, drug_mask, pro_mask, Wq, Wk, Wv, Wo, bo, gamma, beta,
           _trace=False, **extra):
    global _NC
    if _NC is None:
        _NC = build_bass()
    nc = _NC
    in_maps = []
    for c in range(NCORES):
        sl = slice(c * BPC, (c + 1) * BPC)
        in_maps.append({
            "drug": np.ascontiguousarray(drug[sl], dtype=np.float32),
            "target": np.ascontiguousarray(target[sl], dtype=np.float32),
            "dmask": np.ascontiguousarray(drug_mask[sl], dtype=np.int32),
            "pmask": np.ascontiguousarray(pro_mask[sl], dtype=np.int32),
            "Wq": np.ascontiguousarray(Wq, dtype=np.float32),
            "Wk": np.ascontiguousarray(Wk, dtype=np.float32),
            "Wv": np.ascontiguousarray(Wv, dtype=np.float32),
            "Wo": np.ascontiguousarray(Wo, dtype=np.float32),
            "bo": np.ascontiguousarray(bo, dtype=np.float32),
            "gamma": np.ascontiguousarray(gamma, dtype=np.float32),
            "beta": np.ascontiguousarray(beta, dtype=np.float32),
        })
    kw = {}
    if _trace:
        kw = dict(trace=True, trace_cores=[0])
    r_all = run_bass_kernel_spmd(nc, in_maps, list(range(NCORES)), **kw)
    res = r_all.results
    out = np.concatenate([r["out"] for r in res], axis=0)
    attn = np.concatenate([r["attn"] for r in res], axis=0)
    if _trace:
        return out, attn, r_all
    return out, attn
